# revision 16
# baseline (speedup 1.0000x reference)
"""NF4-style 4-bit quantized linear: out = x @ dequant(w).T on 8 TRN2 NeuronCores.

Column-parallel sharding: core c owns output features [c*512, (c+1)*512) and the
matching slices of the packed weight + quant state arrays. x is replicated, but
pre-transposed (and k-permuted to match nibble order) on the host so the kernel
never needs xbar transposes for x:

  xT[r, t] with r = 256*g + 128*e + j  <->  k = 256*g + 2*j + e
    (g = 256-k group = byte-tile, e = nibble parity, j = byte-within-tile)

Per core:
  1. quant scales S = (absmax/code)*(absmax2/code2) and offS = offset*S are
     computed on-chip into one [of, 128] fp16 DRAM tensor (S in cols 0:64,
     offS in 64:128), transpose-loaded to [128, of], then broadcast to
     [128 byte, of] tiles (32 partitions per block row) with tiny PE matmuls
     against 0/1 indicator matrices.
  2. packed bytes (int32 viewed as uint16 pairs, low half = byte value) are
     loaded TRANSPOSED via strided 2-byte xbar DMAs giving [byte, of] tiles;
     nibble extraction yields the even-k and odd-k weight k-tiles
     [128 k, 512 of] directly in matmul layout - no weight round-trip.
     All-16-bit dequant is split across DVE and GPSIMD.
  3. fp16 matmuls accumulate over 32 k-tiles into PSUM [128 tok, 512 of];
     ACT evicts psum->sbuf fp16, sync-queue DMAs store to DRAM.
Host gathers the per-core [8192, 512] outputs with a concat along axis 1.
"""
import numpy as np

import concourse.bass as bass
import concourse.mybir as mybir
import concourse.tile as tile
from concourse import bacc
from concourse.tile_rust import add_dep_helper as tile_rust_add_dep
from concourse.bass_utils import run_bass_kernel_spmd

F16 = mybir.dt.float16
F32 = mybir.dt.float32
I32 = mybir.dt.int32
I16 = mybir.dt.int16
U16 = mybir.dt.uint16
Alu = mybir.AluOpType

P = 128
TOKENS = 8192
IN_F = 4096
OUT_F = 4096
N_CORES = 8
O_C = OUT_F // N_CORES          # 512 out features per core
KT = IN_F // P                  # 32 k-tiles
BPR = IN_F // 2                 # 2048 packed bytes per weight row
NBT = KT // 2                   # 16 byte-tiles (128 bytes = 256 k each)
TOK_BLK = 256                   # tokens per x block
NBLK = TOKENS // TOK_BLK        # 32 x blocks
NTT = TOKENS // P               # 64 token-tiles
RAMP_TT = 4                     # token-tiles processed k-tile-major during ramp


def _build(tokens=TOKENS):
    nc = bacc.Bacc("TRN2", target_bir_lowering=False, debug=False,
                   enable_asserts=False)

    x = nc.dram_tensor("x", [IN_F, tokens], F16, kind="ExternalInput").ap()
    qw = nc.dram_tensor("qw", [O_C, BPR], U16, kind="ExternalInput").ap()
    qam = nc.dram_tensor("qam", [O_C, 64], I32, kind="ExternalInput").ap()
    qcode = nc.dram_tensor("qcode", [O_C, 64], F32, kind="ExternalInput").ap()
    qoff = nc.dram_tensor("qoff", [O_C, 64], F32, kind="ExternalInput").ap()
    am2 = nc.dram_tensor("am2", [O_C, 16], F32, kind="ExternalInput").ap()
    c2 = nc.dram_tensor("c2", [O_C, 16], F32, kind="ExternalInput").ap()
    bmat = nc.dram_tensor("bmat", [2 * NBT * P, P], F16,
                          kind="ExternalInput").ap()
    out = nc.dram_tensor("out", [tokens, O_C], F16, kind="ExternalOutput").ap()
    dbgS = nc.dram_tensor("dbgS", [P, O_C], F16, kind="ExternalOutput").ap()
    dbgW = nc.dram_tensor("dbgW", [P, O_C], F16, kind="ExternalOutput").ap()

    xv = x.rearrange("(a p) t -> p a t", p=P)       # [128, 32, tokens]

    with tile.TileContext(nc) as tc:
        with tc.tile_pool(name="wt_pool", bufs=1) as wt_pool, \
             tc.tile_pool(name="sdr", bufs=1, space="DRAM") as sdr, \
             tc.tile_pool(name="sc_pool", bufs=1) as sc_pool, \
             tc.tile_pool(name="sb_pool", bufs=4) as sb_pool, \
             tc.tile_pool(name="ob_pool", bufs=4) as ob_pool, \
             tc.tile_pool(name="qt_pool", bufs=6) as qt_pool, \
             tc.tile_pool(name="dq", bufs=3) as dq, \
             tc.tile_pool(name="xb_pool", bufs=4) as xb_pool, \
             tc.tile_pool(name="ps_pool", bufs=8, space="PSUM") as ps_pool, \
             tc.tile_pool(name="ev_pool", bufs=4) as ev_pool:

            wts = [wt_pool.tile([P, O_C], F16, name=f"wt{kt}")
                   for kt in range(KT)]

            # ---- quant scale prep: S = (am/code)*(am2/c2), offS = off*S ----
            am3 = sc_pool.tile([P, 4, 64], F32, name="am3")
            nc.gpsimd.dma_start(am3, qam.rearrange("(a p) c -> p a c", p=P))
            cd3 = sc_pool.tile([P, 4, 64], F32, name="cd3")
            nc.gpsimd.dma_start(cd3, qcode.rearrange("(a p) c -> p a c", p=P))
            of3 = sc_pool.tile([P, 4, 64], F32, name="of3")
            nc.gpsimd.dma_start(of3, qoff.rearrange("(a p) c -> p a c", p=P))
            am23 = sc_pool.tile([P, 4, 16], F32, name="am23")
            nc.gpsimd.dma_start(am23, am2.rearrange("(a p) c -> p a c", p=P))
            c23 = sc_pool.tile([P, 4, 16], F32, name="c23")
            nc.gpsimd.dma_start(c23, c2.rearrange("(a p) c -> p a c", p=P))

            rc = sc_pool.tile([P, 4, 64], F32, name="rc")
            nc.vector.reciprocal(rc, cd3)
            s1 = sc_pool.tile([P, 4, 64], F32, name="s1")
            nc.vector.tensor_tensor(s1, am3, rc, Alu.mult)
            rc2 = sc_pool.tile([P, 4, 16], F32, name="rc2")
            nc.vector.reciprocal(rc2, c23)
            s2 = sc_pool.tile([P, 4, 16], F32, name="s2")
            nc.vector.tensor_tensor(s2, am23, rc2, Alu.mult)
            S3f = sc_pool.tile([P, 4, 64], F32, name="S3f")
            nc.vector.tensor_tensor(
                S3f, s1, s2.unsqueeze(3).broadcast_to([P, 4, 16, 4]), Alu.mult)
            S316 = sc_pool.tile([P, 4, 64], F16, name="S316")
            nc.vector.tensor_copy(S316, S3f)
            O316 = sc_pool.tile([P, 4, 64], F16, name="O316")
            nc.vector.tensor_tensor(O316, of3, S3f, Alu.mult)

            # S into cols 0:64, offS into cols 64:128 of one [of, 128] tensor
            sodram = sdr.tile([O_C, P], F16, name="sodram")
            nc.gpsimd.dma_start(
                sodram[:, 0:64].rearrange("(a p) c -> p a c", p=P), S316)
            nc.gpsimd.dma_start(
                sodram[:, 64:128].rearrange("(a p) c -> p a c", p=P), O316)

            # transpose to [128 rows, of]: rows 0:64 = S blocks, 64:128 = offS
            SO_T = sc_pool.tile([P, O_C], F16, name="SO_T")
            ti_so = nc.scalar.dma_start(out=SO_T, in_=sodram[:, :],
                                        transpose=True)

            # indicator matrices (host constants):
            #   B_s[bt][r, j] = (r == 4*bt + j//32)
            #   B_o[bt][r, j] = (r == 64 + 4*bt + j//32)
            Bs, Bo = [], []
            for bt in range(NBT):
                b = sc_pool.tile([P, P], F16, name=f"Bs{bt}")
                nc.gpsimd.dma_start(b, bmat[2 * bt * P:(2 * bt + 1) * P, :])
                Bs.append(b)
                b = sc_pool.tile([P, P], F16, name=f"Bo{bt}")
                nc.gpsimd.dma_start(b, bmat[(2 * bt + 1) * P:(2 * bt + 2) * P, :])
                Bo.append(b)

            # ---- packed-byte transposed loads (strided 2-byte xbar) ----
            qts = {}
            qt_chain = [ti_so]

            def load_qt(bt):
                qt = qt_pool.tile([P, O_C], U16, name="qt")
                b0 = P * bt
                ti = nc.scalar.dma_start(out=qt, in_=qw[:, b0:b0 + P],
                                         transpose=True)
                qt_chain.append(ti)
                qts[bt] = qt

            # ---- scale broadcast via PE: S_b[p, o] = S(block 4bt+p//32, o) --
            S_b, O_b = {}, {}

            def emit_bcast(bt):
                psS = ps_pool.tile([P, O_C], F32, name="ps")
                nc.tensor.matmul(psS, Bs[bt], SO_T)
                sb = sb_pool.tile([P, O_C], F16, name="sb")
                nc.scalar.copy(sb, psS)
                S_b[bt] = sb
                psO = ps_pool.tile([P, O_C], F32, name="ps")
                nc.tensor.matmul(psO, Bo[bt], SO_T)
                obt = ob_pool.tile([P, O_C], F16, name="obt")
                nc.scalar.copy(obt, psO)
                O_b[bt] = obt

            # ---- dequant: k-tiles [128 k, 512 of] fp16, split DVE/GPSIMD --
            def dequant(bt):
                qt = qts.pop(bt)
                sb, obt = S_b.pop(bt), O_b.pop(bt)
                hi = dq.tile([P, O_C], U16, name="hi")
                nc.vector.tensor_scalar(hi, qt, 4, None, Alu.logical_shift_right)
                lo = dq.tile([P, O_C], F16, name="lo")
                nc.vector.scalar_tensor_tensor(lo, hi, -16.0, qt,
                                               Alu.mult, Alu.add)
                wlo = dq.tile([P, O_C], F16, name="wlo")
                nc.vector.tensor_tensor(wlo, lo, sb, Alu.mult)
                whi = dq.tile([P, O_C], F16, name="whi")
                nc.vector.tensor_tensor(whi, hi, sb, Alu.mult)
                nc.vector.tensor_tensor(wts[2 * bt], wlo, obt, Alu.subtract)
                nc.vector.tensor_tensor(wts[2 * bt + 1], whi, obt, Alu.subtract)

            # ---- x loads: plain DMAs on the sync queue ----
            ntt = tokens // P
            nblk = tokens // TOK_BLK
            ramp_tt = min(RAMP_TT, ntt)
            # ramp blocks 0/1 split into 8-ktile chunks for early release
            xramp = {}
            for blk in range(ramp_tt // 2):
                for c in range(4):
                    t = xb_pool.tile([P, 8, TOK_BLK], F16,
                                     name=f"xr{blk}_{c}", bufs=1)
                    nc.sync.dma_start(t, xv[:, 8 * c:8 * c + 8,
                                            blk * TOK_BLK:(blk + 1) * TOK_BLK])
                    xramp[(blk, c)] = t

            xblks = {}

            def load_xblk(blk):
                t = xb_pool.tile([P, KT, TOK_BLK], F16, name="xb")
                nc.sync.dma_start(
                    t, xv[:, :, blk * TOK_BLK:(blk + 1) * TOK_BLK])
                xblks[blk] = t

            # ---- emission: qt prefetch, broadcasts, dequant stream ----
            load_qt(0)
            load_qt(1)
            for bt in range(4):
                emit_bcast(bt)
            dbg_sb = S_b[0]
            for bt in range(NBT):
                if bt + 2 < NBT:
                    load_qt(bt + 2)
                if bt + 4 < NBT:
                    emit_bcast(bt + 4)
                dequant(bt)

            # debug taps (first S-broadcast tile + first weight k-tile)
            nc.sync.dma_start(dbgS, dbg_sb)
            nc.sync.dma_start(dbgW, wts[0])

            # ---- phase A: first ramp_tt token-tiles, k-tile-major ----
            psA = [ps_pool.tile([P, O_C], F32, name="ps")
                   for tt in range(ramp_tt)]
            for kt in range(KT):
                for tt in range(ramp_tt):
                    blk, sub = tt // 2, tt % 2
                    xap = xramp[(blk, kt // 8)][:, kt % 8,
                                                sub * P:(sub + 1) * P]
                    nc.tensor.matmul(psA[tt], xap, wts[kt],
                                     start=(kt == 0), stop=(kt == KT - 1))
            for tt in range(ramp_tt):
                ev = ev_pool.tile([P, O_C], F16, name="ev")
                nc.scalar.copy(ev, psA[tt])
                nc.sync.dma_start(out[tt * P:(tt + 1) * P, :], ev)

            # ---- phase B: remaining token-tiles, token-major ----
            for blk in range(ramp_tt // 2, min(ramp_tt // 2 + 4, nblk)):
                load_xblk(blk)
            for tt in range(ramp_tt, ntt):
                blk, sub = tt // 2, tt % 2
                if sub == 0 and blk + 4 < nblk:
                    load_xblk(blk + 4)
                xt = xblks[blk]
                ps = ps_pool.tile([P, O_C], F32, name="ps")
                for kt in range(KT):
                    nc.tensor.matmul(ps, xt[:, kt, sub * P:(sub + 1) * P],
                                     wts[kt], start=(kt == 0),
                                     stop=(kt == KT - 1))
                ev = ev_pool.tile([P, O_C], F16, name="ev")
                nc.scalar.copy(ev, ps)
                nc.sync.dma_start(out[tt * P:(tt + 1) * P, :], ev)

            # ---- pin xbar/scalar-queue DMA order ----
            for a, b in zip(qt_chain[1:], qt_chain):
                tile_rust_add_dep(a.ins, b.ins, True, "xbar order")

    nc.compile()
    return nc


_NC_CACHE = {}


def _get_nc(tokens=TOKENS):
    if tokens not in _NC_CACHE:
        _NC_CACHE[tokens] = _build(tokens)
    return _NC_CACHE[tokens]


def _shard(inputs):
    x = np.asarray(inputs["x"], dtype=np.float16)
    qw = np.asarray(inputs["quantized_weight"], dtype=np.int32)
    qam = np.asarray(inputs["quant_absmax"], dtype=np.int32)
    qcode = np.asarray(inputs["quant_code"], dtype=np.float32)
    qoff = np.asarray(inputs["quant_offset"], dtype=np.float32)
    am2 = np.asarray(inputs["state2_absmax"], dtype=np.float32)
    c2 = np.asarray(inputs["state2_code"], dtype=np.float32)

    # Pre-transpose + k-permute x on the host (sharding-layout choice):
    # row r = 256*g + 128*e + j  holds  x[:, 256*g + 2*j + e]
    xr = x.reshape(TOKENS, NBT, P, 2)
    xT = np.ascontiguousarray(
        xr.transpose(1, 3, 2, 0).reshape(IN_F, TOKENS))

    # indicator matrices for the PE scale broadcast (constant, replicated)
    r = np.arange(P)[:, None]
    j32 = np.arange(P)[None, :] // 32
    bmat = np.zeros((2 * NBT * P, P), dtype=np.float16)
    for bt in range(NBT):
        bmat[2 * bt * P:(2 * bt + 1) * P] = (r == 4 * bt + j32)
        bmat[(2 * bt + 1) * P:(2 * bt + 2) * P] = (r == 64 + 4 * bt + j32)

    pb = O_C * BPR        # packed bytes per core
    nb1 = O_C * 64        # primary blocks per core
    nb2 = O_C * 16        # secondary blocks per core
    in_maps = []
    for c in range(N_CORES):
        in_maps.append({
            "x": xT,
            "bmat": bmat,
            # packed byte values as uint16 (value-preserving cast)
            "qw": np.ascontiguousarray(
                qw[c * pb:(c + 1) * pb].reshape(O_C, BPR).astype(np.uint16)),
            "qam": np.ascontiguousarray(
                qam[c * nb1:(c + 1) * nb1].reshape(O_C, 64)),
            "qcode": np.ascontiguousarray(
                qcode[c * nb1:(c + 1) * nb1].reshape(O_C, 64)),
            "qoff": np.ascontiguousarray(
                qoff[c * nb1:(c + 1) * nb1].reshape(O_C, 64)),
            "am2": np.ascontiguousarray(
                am2[c * nb2:(c + 1) * nb2].reshape(O_C, 16)),
            "c2": np.ascontiguousarray(
                c2[c * nb2:(c + 1) * nb2].reshape(O_C, 16)),
        })
    return in_maps


def _run(inputs, trace=False, trace_cores=None):
    nc = _get_nc()
    in_maps = _shard(inputs)
    res = run_bass_kernel_spmd(
        nc, in_maps, list(range(N_CORES)), trace=trace,
        trace_cores=trace_cores)
    out = np.concatenate([r["out"] for r in res.results], axis=1)
    return out, res


def kernel(**inputs) -> np.ndarray:
    out, _ = _run(inputs, trace=False)
    return out


# revision 21
# speedup vs baseline: 1.0484x; 1.0484x over previous
"""NF4-style 4-bit quantized linear: out = x @ dequant(w).T on 8 TRN2 NeuronCores.

Column-parallel sharding: core c owns output features [c*512, (c+1)*512) and the
matching slices of the packed weight + quant state arrays. x is replicated, but
pre-transposed (and k-permuted to match nibble order) on the host so the kernel
never needs xbar transposes for x:

  xT[r, t] with r = 256*g + 128*e + j  <->  k = 256*g + 2*j + e
    (g = 256-k group = byte-tile, e = nibble parity, j = byte-within-tile)

Per core:
  1. quant scales S = (absmax/code)*(absmax2/code2) and offS = offset*S are
     computed on-chip into one [of, 128] fp16 DRAM tensor (S in cols 0:64,
     offS in 64:128), transpose-loaded to [128, of], then broadcast to
     [128 byte, of] tiles (32 partitions per block row) with tiny PE matmuls
     against 0/1 indicator matrices.
  2. packed bytes (int32 viewed as uint16 pairs, low half = byte value) are
     loaded TRANSPOSED via strided 2-byte xbar DMAs giving [byte, of] tiles;
     nibble extraction yields the even-k and odd-k weight k-tiles
     [128 k, 512 of] directly in matmul layout - no weight round-trip.
     All-16-bit dequant is split across DVE and GPSIMD.
  3. fp16 matmuls accumulate over 32 k-tiles into PSUM [128 tok, 512 of];
     ACT evicts psum->sbuf fp16, sync-queue DMAs store to DRAM.
Host gathers the per-core [8192, 512] outputs with a concat along axis 1.
"""
import numpy as np

import concourse.bass as bass
import concourse.mybir as mybir
import concourse.tile as tile
from concourse import bacc
from concourse.tile_rust import add_dep_helper as tile_rust_add_dep
from concourse.bass_utils import run_bass_kernel_spmd

F16 = mybir.dt.float16
F32 = mybir.dt.float32
I32 = mybir.dt.int32
I16 = mybir.dt.int16
U16 = mybir.dt.uint16
Alu = mybir.AluOpType

P = 128
TOKENS = 8192
IN_F = 4096
OUT_F = 4096
N_CORES = 8
O_C = OUT_F // N_CORES          # 512 out features per core
KT = IN_F // P                  # 32 k-tiles
BPR = IN_F // 2                 # 2048 packed bytes per weight row
NBT = KT // 2                   # 16 byte-tiles (128 bytes = 256 k each)
TOK_BLK = 256                   # tokens per x block
NBLK = TOKENS // TOK_BLK        # 32 x blocks
NTT = TOKENS // P               # 64 token-tiles
RAMP_TT = 4                     # token-tiles processed k-tile-major during ramp


def _build(tokens=TOKENS):
    nc = bacc.Bacc("TRN2", target_bir_lowering=False, debug=False,
                   enable_asserts=False)

    x = nc.dram_tensor("x", [IN_F, tokens], F16, kind="ExternalInput").ap()
    qw = nc.dram_tensor("qw", [O_C, BPR], U16, kind="ExternalInput").ap()
    qam = nc.dram_tensor("qam", [O_C, 64], I32, kind="ExternalInput").ap()
    qcode = nc.dram_tensor("qcode", [O_C, 64], F32, kind="ExternalInput").ap()
    qoff = nc.dram_tensor("qoff", [O_C, 64], F32, kind="ExternalInput").ap()
    am2 = nc.dram_tensor("am2", [O_C, 16], F32, kind="ExternalInput").ap()
    c2 = nc.dram_tensor("c2", [O_C, 16], F32, kind="ExternalInput").ap()
    bmat = nc.dram_tensor("bmat", [2 * NBT * P, P], F16,
                          kind="ExternalInput").ap()
    out = nc.dram_tensor("out", [tokens, O_C], F16, kind="ExternalOutput").ap()
    dbgS = nc.dram_tensor("dbgS", [P, O_C], F16, kind="ExternalOutput").ap()
    dbgW = nc.dram_tensor("dbgW", [P, O_C], F16, kind="ExternalOutput").ap()

    xv = x.rearrange("(a p) t -> p a t", p=P)       # [128, 32, tokens]

    with tile.TileContext(nc) as tc:
        with tc.tile_pool(name="wt_pool", bufs=1) as wt_pool, \
             tc.tile_pool(name="sdr", bufs=1, space="DRAM") as sdr, \
             tc.tile_pool(name="sc_pool", bufs=1) as sc_pool, \
             tc.tile_pool(name="sb_pool", bufs=4) as sb_pool, \
             tc.tile_pool(name="ob_pool", bufs=4) as ob_pool, \
             tc.tile_pool(name="qt_pool", bufs=1) as qt_pool, \
             tc.tile_pool(name="dq", bufs=3) as dq, \
             tc.tile_pool(name="xb_pool", bufs=4) as xb_pool, \
             tc.tile_pool(name="ps_pool", bufs=8, space="PSUM") as ps_pool, \
             tc.tile_pool(name="ev_pool", bufs=4) as ev_pool:

            wts = [wt_pool.tile([P, O_C], F16, name=f"wt{kt}")
                   for kt in range(KT)]

            # ---- packed-byte transposed loads: 4 byte-tiles per 2-byte
            # xbar transpose; out[p, a, of] = qw[of, 512*i + a*128 + p]
            qt4 = []
            qt_chain = []

            def load_qt4(i):
                qt = qt_pool.tile([P, 4, O_C], U16, name=f"qt{i}")
                ti = nc.scalar.dma_start(out=qt, in_=qw[:, 512 * i:512 * (i + 1)],
                                         transpose=True)
                qt_chain.append(ti)
                qt4.append(qt)

            load_qt4(0)

            # ---- quant scale prep: S = (am/code)*(am2/c2), offS = off*S ----
            am3 = sc_pool.tile([P, 4, 64], F32, name="am3")
            nc.gpsimd.dma_start(am3, qam.rearrange("(a p) c -> p a c", p=P))
            cd3 = sc_pool.tile([P, 4, 64], F32, name="cd3")
            nc.gpsimd.dma_start(cd3, qcode.rearrange("(a p) c -> p a c", p=P))
            of3 = sc_pool.tile([P, 4, 64], F32, name="of3")
            nc.gpsimd.dma_start(of3, qoff.rearrange("(a p) c -> p a c", p=P))
            am23 = sc_pool.tile([P, 4, 16], F32, name="am23")
            nc.gpsimd.dma_start(am23, am2.rearrange("(a p) c -> p a c", p=P))
            c23 = sc_pool.tile([P, 4, 16], F32, name="c23")
            nc.gpsimd.dma_start(c23, c2.rearrange("(a p) c -> p a c", p=P))

            rc = sc_pool.tile([P, 4, 64], F32, name="rc")
            nc.vector.reciprocal(rc, cd3)
            s1 = sc_pool.tile([P, 4, 64], F32, name="s1")
            nc.vector.tensor_tensor(s1, am3, rc, Alu.mult)
            rc2 = sc_pool.tile([P, 4, 16], F32, name="rc2")
            nc.vector.reciprocal(rc2, c23)
            s2 = sc_pool.tile([P, 4, 16], F32, name="s2")
            nc.vector.tensor_tensor(s2, am23, rc2, Alu.mult)
            S3f = sc_pool.tile([P, 4, 64], F32, name="S3f")
            nc.vector.tensor_tensor(
                S3f, s1, s2.unsqueeze(3).broadcast_to([P, 4, 16, 4]), Alu.mult)
            S316 = sc_pool.tile([P, 4, 64], F16, name="S316")
            nc.vector.tensor_copy(S316, S3f)
            O316 = sc_pool.tile([P, 4, 64], F16, name="O316")
            nc.vector.tensor_tensor(O316, of3, S3f, Alu.mult)

            # indicator matrices (host constants), one DMA:
            #   Ball[:, 2*bt, j] row r: (r == 4*bt + j//32)   (S select)
            #   Ball[:, 2*bt+1, j]:     (r == 64 + 4*bt + j//32) (offS select)
            Ball = sc_pool.tile([P, 2 * NBT, P], F16, name="Ball")
            nc.gpsimd.dma_start(Ball, bmat.rearrange("(a p) c -> p a c", p=P))

            # S into cols 0:64, offS into cols 64:128 of one [of, 128] tensor
            sodram = sdr.tile([O_C, P], F16, name="sodram")
            nc.gpsimd.dma_start(
                sodram[:, 0:64].rearrange("(a p) c -> p a c", p=P), S316)
            nc.gpsimd.dma_start(
                sodram[:, 64:128].rearrange("(a p) c -> p a c", p=P), O316)

            # transpose to [128 rows, of]: rows 0:64 = S blocks, 64:128 = offS
            SO_T = sc_pool.tile([P, O_C], F16, name="SO_T")
            ti_so = nc.scalar.dma_start(out=SO_T, in_=sodram[:, :],
                                        transpose=True)
            qt_chain.append(ti_so)
            for i in range(1, 4):
                load_qt4(i)

            # ---- scale broadcast via PE: S_b[p, o] = S(block 4bt+p//32, o) --
            S_b, O_b = {}, {}

            def emit_bcast(bt):
                psS = ps_pool.tile([P, O_C], F32, name="ps")
                nc.tensor.matmul(psS, Ball[:, 2 * bt, :], SO_T)
                sb = sb_pool.tile([P, O_C], F16, name="sb")
                nc.scalar.copy(sb, psS)
                S_b[bt] = sb
                psO = ps_pool.tile([P, O_C], F32, name="ps")
                nc.tensor.matmul(psO, Ball[:, 2 * bt + 1, :], SO_T)
                obt = ob_pool.tile([P, O_C], F16, name="obt")
                nc.scalar.copy(obt, psO)
                O_b[bt] = obt

            # ---- dequant: k-tiles [128 k, 512 of] fp16, all on DVE ----
            def dequant(bt):
                qt = qt4[bt // 4][:, bt % 4, :]
                sb, obt = S_b.pop(bt), O_b.pop(bt)
                hi = dq.tile([P, O_C], U16, name="hi")
                nc.vector.tensor_scalar(hi, qt, 4, None, Alu.logical_shift_right)
                lo = dq.tile([P, O_C], U16, name="lo")
                nc.vector.tensor_scalar(lo, qt, 15, None, Alu.bitwise_and)
                wlo = dq.tile([P, O_C], F16, name="wlo")
                nc.vector.tensor_tensor(wlo, lo, sb, Alu.mult)
                whi = dq.tile([P, O_C], F16, name="whi")
                nc.vector.tensor_tensor(whi, hi, sb, Alu.mult)
                nc.vector.tensor_tensor(wts[2 * bt], wlo, obt, Alu.subtract)
                nc.vector.tensor_tensor(wts[2 * bt + 1], whi, obt, Alu.subtract)

            # ---- x loads: plain DMAs on the sync queue ----
            ntt = tokens // P
            nblk = tokens // TOK_BLK
            ramp_tt = min(RAMP_TT, ntt)
            # ramp blocks 0/1 split into 8-ktile chunks for early release
            xramp = {}
            for c in range(4):
                for blk in range(ramp_tt // 2):
                    t = xb_pool.tile([P, 8, TOK_BLK], F16,
                                     name=f"xr{blk}_{c}", bufs=1)
                    nc.sync.dma_start(t, xv[:, 8 * c:8 * c + 8,
                                            blk * TOK_BLK:(blk + 1) * TOK_BLK])
                    xramp[(blk, c)] = t

            xblks = {}

            def load_xblk(blk):
                t = xb_pool.tile([P, KT, TOK_BLK], F16, name="xb")
                nc.sync.dma_start(
                    t, xv[:, :, blk * TOK_BLK:(blk + 1) * TOK_BLK])
                xblks[blk] = t

            # ---- emission: broadcasts 2 ahead of the dequant stream ----
            emit_bcast(0)
            emit_bcast(1)
            dbg_sb = S_b[0]
            for bt in range(NBT):
                if bt + 2 < NBT:
                    emit_bcast(bt + 2)
                dequant(bt)

            # debug taps (first S-broadcast tile + first weight k-tile)
            nc.gpsimd.dma_start(dbgS, dbg_sb)
            nc.gpsimd.dma_start(dbgW, wts[0])

            # ---- phase A: first ramp_tt token-tiles, k-tile-major ----
            psA = [ps_pool.tile([P, O_C], F32, name="ps")
                   for tt in range(ramp_tt)]
            for kt in range(KT):
                for tt in range(ramp_tt):
                    blk, sub = tt // 2, tt % 2
                    xap = xramp[(blk, kt // 8)][:, kt % 8,
                                                sub * P:(sub + 1) * P]
                    nc.tensor.matmul(psA[tt], xap, wts[kt],
                                     start=(kt == 0), stop=(kt == KT - 1))
            for tt in range(ramp_tt):
                ev = ev_pool.tile([P, O_C], F16, name="ev")
                nc.scalar.copy(ev, psA[tt])
                nc.sync.dma_start(out[tt * P:(tt + 1) * P, :], ev)

            # ---- phase B: remaining token-tiles, token-major ----
            for blk in range(ramp_tt // 2, min(ramp_tt // 2 + 4, nblk)):
                load_xblk(blk)
            for tt in range(ramp_tt, ntt):
                blk, sub = tt // 2, tt % 2
                if sub == 0 and blk + 4 < nblk:
                    load_xblk(blk + 4)
                xt = xblks[blk]
                ps = ps_pool.tile([P, O_C], F32, name="ps")
                for kt in range(KT):
                    nc.tensor.matmul(ps, xt[:, kt, sub * P:(sub + 1) * P],
                                     wts[kt], start=(kt == 0),
                                     stop=(kt == KT - 1))
                ev = ev_pool.tile([P, O_C], F16, name="ev")
                nc.scalar.copy(ev, ps)
                nc.sync.dma_start(out[tt * P:(tt + 1) * P, :], ev)

            # ---- pin xbar/scalar-queue DMA order ----
            for a, b in zip(qt_chain[1:], qt_chain):
                tile_rust_add_dep(a.ins, b.ins, True, "xbar order")

    nc.compile()
    return nc


_NC_CACHE = {}


def _get_nc(tokens=TOKENS):
    if tokens not in _NC_CACHE:
        _NC_CACHE[tokens] = _build(tokens)
    return _NC_CACHE[tokens]


def _shard(inputs):
    x = np.asarray(inputs["x"], dtype=np.float16)
    qw = np.asarray(inputs["quantized_weight"], dtype=np.int32)
    qam = np.asarray(inputs["quant_absmax"], dtype=np.int32)
    qcode = np.asarray(inputs["quant_code"], dtype=np.float32)
    qoff = np.asarray(inputs["quant_offset"], dtype=np.float32)
    am2 = np.asarray(inputs["state2_absmax"], dtype=np.float32)
    c2 = np.asarray(inputs["state2_code"], dtype=np.float32)

    # Pre-transpose + k-permute x on the host (sharding-layout choice):
    # row r = 256*g + 128*e + j  holds  x[:, 256*g + 2*j + e]
    xr = x.reshape(TOKENS, NBT, P, 2)
    xT = np.ascontiguousarray(
        xr.transpose(1, 3, 2, 0).reshape(IN_F, TOKENS))

    # indicator matrices for the PE scale broadcast (constant, replicated)
    r = np.arange(P)[:, None]
    j32 = np.arange(P)[None, :] // 32
    bmat = np.zeros((2 * NBT * P, P), dtype=np.float16)
    for bt in range(NBT):
        bmat[2 * bt * P:(2 * bt + 1) * P] = (r == 4 * bt + j32)
        bmat[(2 * bt + 1) * P:(2 * bt + 2) * P] = (r == 64 + 4 * bt + j32)

    pb = O_C * BPR        # packed bytes per core
    nb1 = O_C * 64        # primary blocks per core
    nb2 = O_C * 16        # secondary blocks per core
    in_maps = []
    for c in range(N_CORES):
        in_maps.append({
            "x": xT,
            "bmat": bmat,
            # packed byte values as uint16 (value-preserving cast)
            "qw": np.ascontiguousarray(
                qw[c * pb:(c + 1) * pb].reshape(O_C, BPR).astype(np.uint16)),
            "qam": np.ascontiguousarray(
                qam[c * nb1:(c + 1) * nb1].reshape(O_C, 64)),
            "qcode": np.ascontiguousarray(
                qcode[c * nb1:(c + 1) * nb1].reshape(O_C, 64)),
            "qoff": np.ascontiguousarray(
                qoff[c * nb1:(c + 1) * nb1].reshape(O_C, 64)),
            "am2": np.ascontiguousarray(
                am2[c * nb2:(c + 1) * nb2].reshape(O_C, 16)),
            "c2": np.ascontiguousarray(
                c2[c * nb2:(c + 1) * nb2].reshape(O_C, 16)),
        })
    return in_maps


def _run(inputs, trace=False, trace_cores=None):
    nc = _get_nc()
    in_maps = _shard(inputs)
    res = run_bass_kernel_spmd(
        nc, in_maps, list(range(N_CORES)), trace=trace,
        trace_cores=trace_cores)
    out = np.concatenate([r["out"] for r in res.results], axis=1)
    return out, res


def kernel(**inputs) -> np.ndarray:
    out, _ = _run(inputs, trace=False)
    return out


# revision 29
# speedup vs baseline: 1.1069x; 1.0558x over previous
"""NF4-style 4-bit quantized linear: out = x @ dequant(w).T on 8 TRN2 NeuronCores.

Column-parallel sharding: core c owns output features [c*512, (c+1)*512) and the
matching slices of the packed weight + quant state arrays. x is replicated, but
pre-transposed (and k-permuted to match nibble order) on the host so the kernel
never needs xbar transposes for x:

  xT[r, t] with r = 256*g + 128*e + j  <->  k = 256*g + 2*j + e
    (g = 256-k group = byte-tile, e = nibble parity, j = byte-within-tile)

Per core:
  1. quant scales S = (absmax/code)*(absmax2/code2) and offS = offset*S are
     computed on-chip into one [of, 128] fp16 DRAM tensor (S in cols 0:64,
     offS in 64:128), transpose-loaded to [128, of], then broadcast to
     [128 byte, of] tiles (32 partitions per block row) with tiny PE matmuls
     against 0/1 indicator matrices.
  2. packed bytes (int32 viewed as uint16 pairs, low half = byte value) are
     loaded TRANSPOSED via strided 2-byte xbar DMAs giving [byte, of] tiles;
     nibble extraction yields the even-k and odd-k weight k-tiles
     [128 k, 512 of] directly in matmul layout - no weight round-trip.
     All-16-bit dequant is split across DVE and GPSIMD.
  3. fp16 matmuls accumulate over 32 k-tiles into PSUM [128 tok, 512 of];
     ACT evicts psum->sbuf fp16, sync-queue DMAs store to DRAM.
Host gathers the per-core [8192, 512] outputs with a concat along axis 1.
"""
import numpy as np

import concourse.bass as bass
import concourse.mybir as mybir
import concourse.tile as tile
from concourse import bacc
from concourse.tile_rust import add_dep_helper as tile_rust_add_dep
from concourse.bass_utils import run_bass_kernel_spmd

F16 = mybir.dt.float16
F32 = mybir.dt.float32
I32 = mybir.dt.int32
I16 = mybir.dt.int16
U16 = mybir.dt.uint16
Alu = mybir.AluOpType

P = 128
TOKENS = 8192
IN_F = 4096
OUT_F = 4096
N_CORES = 8
O_C = OUT_F // N_CORES          # 512 out features per core
KT = IN_F // P                  # 32 k-tiles
BPR = IN_F // 2                 # 2048 packed bytes per weight row
NBT = KT // 2                   # 16 byte-tiles (128 bytes = 256 k each)
TOK_BLK = 256                   # tokens per x block
NBLK = TOKENS // TOK_BLK        # 32 x blocks
NTT = TOKENS // P               # 64 token-tiles
RAMP_TT = 4                     # token-tiles processed k-tile-major during ramp


def _build(tokens=TOKENS):
    nc = bacc.Bacc("TRN2", target_bir_lowering=False, debug=False,
                   enable_asserts=False)

    x = nc.dram_tensor("x", [IN_F, tokens], F16, kind="ExternalInput").ap()
    qw = nc.dram_tensor("qw", [O_C, BPR], U16, kind="ExternalInput").ap()
    # quant state, host-transposed to [64 blocks, of] fp32 (am2/c2 repeated 4x)
    qam = nc.dram_tensor("qam", [64, O_C], F32, kind="ExternalInput").ap()
    qcode = nc.dram_tensor("qcode", [64, O_C], F32, kind="ExternalInput").ap()
    qoff = nc.dram_tensor("qoff", [64, O_C], F32, kind="ExternalInput").ap()
    am2 = nc.dram_tensor("am2", [64, O_C], F32, kind="ExternalInput").ap()
    c2 = nc.dram_tensor("c2", [64, O_C], F32, kind="ExternalInput").ap()
    bmat = nc.dram_tensor("bmat", [2 * NBT * P, P], F16,
                          kind="ExternalInput").ap()
    out = nc.dram_tensor("out", [tokens, O_C], F16, kind="ExternalOutput").ap()
    dbgS = nc.dram_tensor("dbgS", [P, O_C], F16, kind="ExternalOutput").ap()
    dbgW = nc.dram_tensor("dbgW", [P, O_C], F16, kind="ExternalOutput").ap()

    xv = x.rearrange("(a p) t -> p a t", p=P)       # [128, 32, tokens]

    with tile.TileContext(nc) as tc:
        with tc.tile_pool(name="wt_pool", bufs=1) as wt_pool, \
             tc.tile_pool(name="sc_pool", bufs=1) as sc_pool, \
             tc.tile_pool(name="sb_pool", bufs=4) as sb_pool, \
             tc.tile_pool(name="ob_pool", bufs=4) as ob_pool, \
             tc.tile_pool(name="qt_pool", bufs=1) as qt_pool, \
             tc.tile_pool(name="dq", bufs=3) as dq, \
             tc.tile_pool(name="xb_pool", bufs=4) as xb_pool, \
             tc.tile_pool(name="ps_pool", bufs=8, space="PSUM") as ps_pool, \
             tc.tile_pool(name="ev_pool", bufs=4) as ev_pool:

            wts = [wt_pool.tile([P, O_C], F16, name=f"wt{kt}")
                   for kt in range(KT)]

            # ---- packed-byte transposed loads: 4 byte-tiles per 2-byte
            # xbar transpose; out[p, a, of] = qw[of, 512*i + a*128 + p]
            qt4 = []
            qt_chain = []

            def load_qt4(i):
                qt = qt_pool.tile([P, 4, O_C], U16, name=f"qt{i}")
                ti = nc.scalar.dma_start(out=qt, in_=qw[:, 512 * i:512 * (i + 1)],
                                         transpose=True)
                qt_chain.append(ti)
                qt4.append(qt)

            # ---- PE warmup: keep the clock pstate up while the ramp loads --
            wz = sc_pool.tile([P, P], F16, name="wz")
            nc.vector.memset(wz, 0.0)
            wz5 = sc_pool.tile([P, O_C], F16, name="wz5")
            nc.vector.memset(wz5, 0.0)
            psz = ps_pool.tile([P, O_C], F32, name="ps")
            for i in range(10):
                nc.tensor.matmul(psz, wz, wz5)

            # ---- quant scale prep (already [block, of] layout from host):
            #      SO_T rows 0:64 = S = (am/code)*(am2/c2), 64:128 = off*S
            amT = sc_pool.tile([64, O_C], F32, name="amT")
            nc.scalar.dma_start(amT, qam)
            cdT = sc_pool.tile([64, O_C], F32, name="cdT")
            nc.scalar.dma_start(cdT, qcode)
            ofT = sc_pool.tile([64, O_C], F32, name="ofT")
            nc.scalar.dma_start(ofT, qoff)
            am2T = sc_pool.tile([64, O_C], F32, name="am2T")
            nc.scalar.dma_start(am2T, am2)
            c2T = sc_pool.tile([64, O_C], F32, name="c2T")
            nc.scalar.dma_start(c2T, c2)

            rcT = sc_pool.tile([64, O_C], F32, name="rcT")
            nc.vector.reciprocal(rcT, cdT)
            s1T = sc_pool.tile([64, O_C], F32, name="s1T")
            nc.vector.tensor_tensor(s1T, amT, rcT, Alu.mult)
            rc2T = sc_pool.tile([64, O_C], F32, name="rc2T")
            nc.vector.reciprocal(rc2T, c2T)
            s2T = sc_pool.tile([64, O_C], F32, name="s2T")
            nc.vector.tensor_tensor(s2T, am2T, rc2T, Alu.mult)
            S3fT = sc_pool.tile([64, O_C], F32, name="S3fT")
            nc.vector.tensor_tensor(S3fT, s1T, s2T, Alu.mult)
            SO_T = sc_pool.tile([P, O_C], F16, name="SO_T")
            nc.vector.tensor_copy(SO_T[0:64, :], S3fT)
            nc.vector.tensor_tensor(SO_T[64:128, :], ofT, S3fT, Alu.mult)

            # indicator matrices (host constants), one DMA:
            #   Ball[:, 2*bt, j] row r: (r == 4*bt + j//32)   (S select)
            #   Ball[:, 2*bt+1, j]:     (r == 64 + 4*bt + j//32) (offS select)
            Ball = sc_pool.tile([P, 2 * NBT, P], F16, name="Ball")
            nc.gpsimd.dma_start(Ball, bmat.rearrange("(a p) c -> p a c", p=P))

            for i in range(4):
                load_qt4(i)

            # ---- scale broadcast via PE: S_b[p, o] = S(block 4bt+p//32, o) --
            S_b, O_b = {}, {}

            def emit_bcast(bt):
                psS = ps_pool.tile([P, O_C], F32, name="ps")
                nc.tensor.matmul(psS, Ball[:, 2 * bt, :], SO_T)
                sb = sb_pool.tile([P, O_C], F16, name="sb")
                nc.scalar.copy(sb, psS)
                S_b[bt] = sb
                psO = ps_pool.tile([P, O_C], F32, name="ps")
                nc.tensor.matmul(psO, Ball[:, 2 * bt + 1, :], SO_T)
                obt = ob_pool.tile([P, O_C], F16, name="obt")
                nc.scalar.copy(obt, psO)
                O_b[bt] = obt

            # ---- dequant: k-tiles [128 k, 512 of] fp16, all on DVE ----
            def dequant(bt):
                qt = qt4[bt // 4][:, bt % 4, :]
                sb, obt = S_b.pop(bt), O_b.pop(bt)
                hi = dq.tile([P, O_C], U16, name="hi")
                nc.vector.tensor_scalar(hi, qt, 4, None, Alu.logical_shift_right)
                lo = dq.tile([P, O_C], U16, name="lo")
                nc.vector.tensor_scalar(lo, qt, 15, None, Alu.bitwise_and)
                wlo = dq.tile([P, O_C], F16, name="wlo")
                nc.vector.tensor_tensor(wlo, lo, sb, Alu.mult)
                whi = dq.tile([P, O_C], F16, name="whi")
                nc.vector.tensor_tensor(whi, hi, sb, Alu.mult)
                nc.vector.tensor_tensor(wts[2 * bt], wlo, obt, Alu.subtract)
                nc.vector.tensor_tensor(wts[2 * bt + 1], whi, obt, Alu.subtract)

            # ---- x loads: plain DMAs on the sync queue ----
            ntt = tokens // P
            nblk = tokens // TOK_BLK
            ramp_tt = min(RAMP_TT, ntt)
            # ramp blocks 0/1 split into 8-ktile chunks for early release
            xramp = {}
            xr_insts = []
            for c in range(4):
                for blk in range(ramp_tt // 2):
                    t = xb_pool.tile([P, 8, TOK_BLK], F16,
                                     name=f"xr{blk}_{c}", bufs=1)
                    xi = nc.sync.dma_start(t, xv[:, 8 * c:8 * c + 8,
                                           blk * TOK_BLK:(blk + 1) * TOK_BLK])
                    xramp[(blk, c)] = t
                    xr_insts.append(xi)

            xblks = {}
            xb_insts = []

            def load_xblk(blk):
                t = xb_pool.tile([P, KT, TOK_BLK], F16, name="xb")
                xi = nc.sync.dma_start(
                    t, xv[:, :, blk * TOK_BLK:(blk + 1) * TOK_BLK])
                xblks[blk] = t
                xb_insts.append(xi)

            # ---- emission: broadcasts 2 ahead of the dequant stream ----
            emit_bcast(0)
            emit_bcast(1)
            dbg_sb = S_b[0]
            for bt in range(NBT):
                if bt + 2 < NBT:
                    emit_bcast(bt + 2)
                dequant(bt)

            # debug taps (first S-broadcast tile + first weight k-tile)
            nc.gpsimd.dma_start(dbgS, dbg_sb)
            nc.gpsimd.dma_start(dbgW, wts[0])

            # ---- phase A: first ramp_tt token-tiles, k-tile-major ----
            psA = [ps_pool.tile([P, O_C], F32, name="ps")
                   for tt in range(ramp_tt)]
            for kt in range(KT):
                for tt in range(ramp_tt):
                    blk, sub = tt // 2, tt % 2
                    xap = xramp[(blk, kt // 8)][:, kt % 8,
                                                sub * P:(sub + 1) * P]
                    nc.tensor.matmul(psA[tt], xap, wts[kt],
                                     start=(kt == 0), stop=(kt == KT - 1))
            for tt in range(ramp_tt):
                ev = ev_pool.tile([P, O_C], F16, name="ev")
                nc.scalar.copy(ev, psA[tt])
                nc.sync.dma_start(out[tt * P:(tt + 1) * P, :], ev)

            # ---- phase B: remaining token-tiles, token-major ----
            for blk in range(ramp_tt // 2, min(ramp_tt // 2 + 4, nblk)):
                load_xblk(blk)
            for tt in range(ramp_tt, ntt):
                blk, sub = tt // 2, tt % 2
                if sub == 0 and blk + 4 < nblk:
                    load_xblk(blk + 4)
                xt = xblks[blk]
                ps = ps_pool.tile([P, O_C], F32, name="ps")
                for kt in range(KT):
                    nc.tensor.matmul(ps, xt[:, kt, sub * P:(sub + 1) * P],
                                     wts[kt], start=(kt == 0),
                                     stop=(kt == KT - 1))
                ev = ev_pool.tile([P, O_C], F16, name="ev")
                nc.scalar.copy(ev, ps)
                nc.sync.dma_start(out[tt * P:(tt + 1) * P, :], ev)

            # ---- pin DMA-ring order: zipper the latency-critical transposes
            # with the bulk x loads so big transfers never head-of-line-block
            # the qt/scale path on the shared hardware DMA queues.
            chain = []
            if len(xr_insts) == 8:
                for i in range(4):
                    chain.append(qt_chain[i])
                    chain.append(xr_insts[2 * i])
                    chain.append(xr_insts[2 * i + 1])
            else:
                chain = list(qt_chain) + list(xr_insts)
            chain.extend(xb_insts[:4])
            for a, b in zip(chain[1:], chain):
                tile_rust_add_dep(a.ins, b.ins, True, "dma ring order")

    nc.compile()
    return nc


_NC_CACHE = {}


def _get_nc(tokens=TOKENS):
    if tokens not in _NC_CACHE:
        _NC_CACHE[tokens] = _build(tokens)
    return _NC_CACHE[tokens]


def _shard(inputs):
    x = np.asarray(inputs["x"], dtype=np.float16)
    qw = np.asarray(inputs["quantized_weight"], dtype=np.int32)
    qam = np.asarray(inputs["quant_absmax"], dtype=np.int32)
    qcode = np.asarray(inputs["quant_code"], dtype=np.float32)
    qoff = np.asarray(inputs["quant_offset"], dtype=np.float32)
    am2 = np.asarray(inputs["state2_absmax"], dtype=np.float32)
    c2 = np.asarray(inputs["state2_code"], dtype=np.float32)

    # Pre-transpose + k-permute x on the host (sharding-layout choice):
    # row r = 256*g + 128*e + j  holds  x[:, 256*g + 2*j + e]
    xr = x.reshape(TOKENS, NBT, P, 2)
    xT = np.ascontiguousarray(
        xr.transpose(1, 3, 2, 0).reshape(IN_F, TOKENS))

    # indicator matrices for the PE scale broadcast (constant, replicated)
    r = np.arange(P)[:, None]
    j32 = np.arange(P)[None, :] // 32
    bmat = np.zeros((2 * NBT * P, P), dtype=np.float16)
    for bt in range(NBT):
        bmat[2 * bt * P:(2 * bt + 1) * P] = (r == 4 * bt + j32)
        bmat[(2 * bt + 1) * P:(2 * bt + 2) * P] = (r == 64 + 4 * bt + j32)

    pb = O_C * BPR        # packed bytes per core
    nb1 = O_C * 64        # primary blocks per core
    nb2 = O_C * 16        # secondary blocks per core
    in_maps = []
    for c in range(N_CORES):
        in_maps.append({
            "x": xT,
            "bmat": bmat,
            # packed byte values as uint16 (value-preserving cast)
            "qw": np.ascontiguousarray(
                qw[c * pb:(c + 1) * pb].reshape(O_C, BPR).astype(np.uint16)),
            "qam": np.ascontiguousarray(
                qam[c * nb1:(c + 1) * nb1].reshape(O_C, 64)
                .T.astype(np.float32)),
            "qcode": np.ascontiguousarray(
                qcode[c * nb1:(c + 1) * nb1].reshape(O_C, 64).T),
            "qoff": np.ascontiguousarray(
                qoff[c * nb1:(c + 1) * nb1].reshape(O_C, 64).T),
            "am2": np.ascontiguousarray(np.repeat(
                am2[c * nb2:(c + 1) * nb2].reshape(O_C, 16).T, 4, axis=0)),
            "c2": np.ascontiguousarray(np.repeat(
                c2[c * nb2:(c + 1) * nb2].reshape(O_C, 16).T, 4, axis=0)),
        })
    return in_maps


def _run(inputs, trace=False, trace_cores=None):
    nc = _get_nc()
    in_maps = _shard(inputs)
    res = run_bass_kernel_spmd(
        nc, in_maps, list(range(N_CORES)), trace=trace,
        trace_cores=trace_cores)
    out = np.concatenate([r["out"] for r in res.results], axis=1)
    return out, res


def kernel(**inputs) -> np.ndarray:
    out, _ = _run(inputs, trace=False)
    return out
